# revision 21
# baseline (speedup 1.0000x reference)
"""GATv2 attention-score kernel for 8 Trainium2 NeuronCores.

Reference computation (per b, h):
    scores[i, j] = sum_d silu(q[i, d] + k[j, d]) * a[h, d]
    attn = softmax(where(mask, -FMAX, scores), axis=-1), zeroed at mask.

Sharding: the 32 (b, h) pairs are split 4-per-core (all four share one b,
so the mask is per-core constant).

Per-core dataflow (all shapes hardcoded: B=4, H=8, LQ=LK=256, D=64):
  - SBUF "summed" tile T (128, 8192): partitions = [d; d] (two stacked
    64-deep d blocks, one per query of a pair), free = 32 query-pairs x
    256 j.  Built with one DVE tensor_scalar_add per pair: in0 = kk
    (128, 256) = [k^T; k^T], scalar = per-partition column of qq
    (q values for the pair), which runs in the DVE 2x_2P mode.
  - ScalarE Silu over the whole tile (the compute floor of the problem:
    4 * 256 * 256 * 64 = 16.8M activations per core).
  - TensorE reduction over d: per pair one matmul with a sparse
    (128, 32) weight holding a_h in rows 0:64 of column 2m and rows
    64:128 of column 2m+1; 16 pair-matmuls accumulate into one
    32-partition PSUM strip (tile_position=(0, 32g)), four strips fill
    a (128, 256) PSUM tile = scores for one i-tile.
  - DVE evacuates PSUM + maskMIN (premasked float32.min addend) into an
    SBUF scores strip; after all 4 (b,h): one ScalarE Exp (masked
    entries underflow to exactly 0), DVE segment reduce_sum +
    reciprocal + per-segment scale, DMA out.

Softmax skips the max-subtraction: scores are bounded (|s| < ~60) so
exp cannot overflow, and masked entries are exactly 0.  Fully-masked
rows would yield NaN but do not occur (P ~ 2^-256 per row).

Precision (PREC below): the summed/silu stage runs in fp16 by default
(k, summed tile, reduction weights fp16; q enters as an exact fp32
per-partition scalar; PSUM/scores/softmax all fp32).  Measured on HW:
  PREC="fp16":  ~110-116 us/core/iter, max rel err 9.0e-4 (abs 1.5e-5)
  PREC="fp32":  ~199 us,               max rel err 1.3e-5 (abs 2.8e-7)
The kernel is ScalarE-bound; the theoretical silu floor is 109.2 us, so
fp16 is at ~101% of roofline (fp32 pays a DVE build-op-overhead wall).
Set PREC = "fp32" for a bit-conservative run if tighter accuracy is
ever required.
"""

import numpy as np

B, H, L, D = 4, 8, 256, 64
NCORES = 8
BH = 4          # (b, h) pairs per core
NPAIR = 128     # query pairs per (b, h)
TBLK = 64       # query pairs per summed tile (fp16; fp32 uses 32)
FT = TBLK * L   # summed tile free size (16384)
FMIN = np.float32(np.finfo(np.float32).min)

_cache = {}
PREC = "fp16"      # "fp32" | "fp16" | "mixed" — summed/silu stage dtype


def _build_program(reps=1, stages="full", prec="fp32"):
    import concourse.mybir as mybir
    from concourse import bacc
    from concourse.tile import TileContext

    DT = mybir.dt.float32
    HT = mybir.dt.float32 if prec == "fp32" else mybir.dt.float16
    WT = DT if prec in ("fp32", "mixed") else HT    # matmul operand dtype
    nc = bacc.Bacc("TRN2", target_bir_lowering=False, debug=False,
                   num_devices=NCORES)

    kk_d = nc.dram_tensor("kk", [BH, 128, L], HT, kind="ExternalInput")
    qq_d = nc.dram_tensor("qq", [BH, 128, NPAIR], DT, kind="ExternalInput")
    wz_d = nc.dram_tensor("wz", [128, BH * 16 * 32], WT, kind="ExternalInput")
    mm_d = nc.dram_tensor("mm", [128, 2 * L], DT, kind="ExternalInput")
    out_d = nc.dram_tensor("out", [BH, 2, 128, L], DT, kind="ExternalOutput")

    with TileContext(nc) as tc:
        with (
            tc.tile_pool(name="io", bufs=2) as io_pool,
            tc.tile_pool(name="const", bufs=1) as c_pool,
            tc.tile_pool(name="summed", bufs=3) as t_pool,
            tc.tile_pool(name="psum", bufs=4, space="PSUM") as ps_pool,
        ):
            wz_t = c_pool.tile([128, BH * 16 * 32], WT, tag="wz")
            nc.sync.dma_start(wz_t[:], wz_d[:])
            mm_t = c_pool.tile([128, 2 * L], DT, tag="mm")
            nc.sync.dma_start(mm_t[:], mm_d[:])
            scores = c_pool.tile([128, BH * 2 * L], DT, tag="scores")
            sums = c_pool.tile([128, BH * 2], DT, tag="sums")
            recip = c_pool.tile([128, BH * 2], DT, tag="recip")

            for _rep in range(reps):
                for l in range(BH):
                    kk_t = io_pool.tile([128, L], HT, tag="kk")
                    nc.sync.dma_start(kk_t[:], kk_d[l])
                    qq_t = io_pool.tile([128, NPAIR], DT, tag="qq")
                    nc.sync.dma_start(qq_t[:], qq_d[l])

                    tblk = 32 if prec == "fp32" else TBLK
                    ft = tblk * L
                    for it in range(2):      # i-tile = 128 queries
                        ps = ps_pool.tile([128, L], DT, tag="ps")
                        for tt in range(64 // tblk):
                            T = t_pool.tile([128, ft], HT, tag="T", bufs=3)
                            if prec == "mixed":
                                S = t_pool.tile([128, ft], WT, tag="S",
                                                name="S", bufs=1)
                            else:
                                S = T
                            base = it * 64 + tt * tblk  # first pair in tile
                            for blk in range(tblk):
                                c = base + blk
                                nc.vector.tensor_scalar_add(
                                    T[:, blk * L:(blk + 1) * L], kk_t[:],
                                    qq_t[:, c:c + 1])
                            if stages == "build":
                                continue
                            nc.scalar.activation(
                                S[:], T[:],
                                mybir.ActivationFunctionType.Silu)
                            if stages == "silu":
                                continue
                            for blk in range(tblk):
                                lc = tt * tblk + blk
                                g, m = lc // 16, lc % 16
                                nc.tensor.matmul(
                                    ps[32 * g:32 * g + 32, :],
                                    lhsT=wz_t[:, (l * 16 + m) * 32:
                                              (l * 16 + m + 1) * 32],
                                    rhs=S[:, blk * L:(blk + 1) * L],
                                    start=(m == 0), stop=(m == 15),
                                    tile_position=(0, 32 * g))
                        seg = l * 2 + it
                        if stages in ("build", "silu"):
                            continue
                        nc.vector.tensor_tensor(
                            scores[:, seg * L:(seg + 1) * L], ps[:],
                            mm_t[:, it * L:(it + 1) * L], mybir.AluOpType.add)

                if stages in ("build", "silu"):
                    # keep per-rep work observable: flush last T to out
                    if prec == "fp32":
                        nc.sync.dma_start(out_d[0, 0], T[:, :L])
                    else:
                        nc.sync.dma_start(out_d[0, 0, :, :L // 2],
                                          T[:, :L].bitcast(mybir.dt.float32))
                    continue
                if stages == "mm":
                    nc.sync.dma_start(out_d[0, 0], scores[:, :L])
                    continue
                nc.scalar.activation(scores[:], scores[:],
                                     mybir.ActivationFunctionType.Exp)
                nc.vector.reduce_sum(
                    sums[:], scores[:].rearrange("p (s j) -> p s j", j=L),
                    axis=mybir.AxisListType.X)
                nc.vector.reciprocal(recip[:], sums[:])
                for seg in range(BH * 2):
                    nc.vector.tensor_scalar_mul(
                        scores[:, seg * L:(seg + 1) * L],
                        scores[:, seg * L:(seg + 1) * L],
                        recip[:, seg:seg + 1])
                for l in range(BH):
                    for it in range(2):
                        seg = l * 2 + it
                        nc.sync.dma_start(out_d[l, it],
                                          scores[:, seg * L:(seg + 1) * L])

    nc.compile()
    return nc


def _prep_core_inputs(q, k, mask, attention, prec="fp32"):
    """Host-side layout prep: per-core input dicts."""
    ht = np.float32 if prec == "fp32" else np.float16
    wt = np.float32 if prec in ("fp32", "mixed") else np.float16
    q = np.asarray(q, np.float32)
    k = np.asarray(k, np.float32)
    a = np.asarray(attention, np.float32).reshape(H, D)
    mask = np.asarray(mask).reshape(B, L, L)

    in_maps = []
    for core in range(NCORES):
        kk = np.empty((BH, 128, L), ht)
        qq = np.empty((BH, 128, NPAIR), np.float32)
        wz4 = np.zeros((BH, 16, 128, 32), wt)
        for l in range(BH):
            f = 4 * core + l
            b, h = f // H, f % H
            kT = k[b, h].T                      # (D, L)
            kk[l, :64] = kT
            kk[l, 64:] = kT
            qq[l, :64] = q[b, h, 0::2].T        # even queries
            qq[l, 64:] = q[b, h, 1::2].T        # odd queries
            for m in range(16):
                wz4[l, m, :64, 2 * m] = a[h]
                wz4[l, m, 64:, 2 * m + 1] = a[h]
        wz = np.ascontiguousarray(
            wz4.transpose(2, 0, 1, 3).reshape(128, BH * 16 * 32))
        mb = np.where(mask[4 * core // H], FMIN, np.float32(0))
        mm = np.ascontiguousarray(
            np.concatenate([mb[:128], mb[128:]], axis=1).astype(np.float32))
        in_maps.append({"kk": kk, "qq": qq, "wz": wz, "mm": mm})
    return in_maps


def _get_runner(prec=None):
    """Persistent jitted shard_map runner over 8 cores.

    Mirrors concourse.bass2jax.run_bass_via_pjrt but caches the jitted
    callable so repeat kernel() calls skip retracing/recompiling.
    """
    if prec is None:
        prec = PREC
    key = ("runner", prec)
    if key in _cache:
        return _cache[key]

    import jax
    import concourse.mybir as mybir
    from jax.sharding import Mesh, PartitionSpec
    from jax.experimental.shard_map import shard_map
    from concourse import bass2jax

    bass2jax.install_neuronx_cc_hook()
    nc = _build_program(prec=prec)

    part_name = (nc.partition_id_tensor.name
                 if nc.partition_id_tensor else None)
    in_names, out_names, out_avals, zero_outs = [], [], [], []
    for alloc in nc.m.functions[0].allocations:
        if not isinstance(alloc, mybir.MemoryLocationSet):
            continue
        name = alloc.memorylocations[0].name
        if alloc.kind == "ExternalInput":
            if name != part_name:
                in_names.append(name)
        elif alloc.kind == "ExternalOutput":
            shape = tuple(alloc.tensor_shape)
            dtype = mybir.dt.np(alloc.dtype)
            out_names.append(name)
            out_avals.append(jax.core.ShapedArray(shape, dtype))
            zero_outs.append(np.zeros(shape, dtype))
    n_params = len(in_names)
    all_names = in_names + out_names
    if part_name is not None:
        all_names = all_names + [part_name]

    def _body(*args):
        operands = list(args)
        if part_name is not None:
            operands.append(bass2jax.partition_id_tensor())
        return tuple(bass2jax._bass_exec_p.bind(
            *operands,
            out_avals=tuple(out_avals),
            in_names=tuple(all_names),
            out_names=tuple(out_names),
            lowering_input_output_aliases=(),
            sim_require_finite=True,
            sim_require_nnan=True,
            nc=nc,
        ))

    devices = jax.devices()[:NCORES]
    mesh = Mesh(np.asarray(devices), ("core",))
    n_outs = len(out_names)
    sharded = jax.jit(
        shard_map(_body, mesh=mesh,
                  in_specs=(PartitionSpec("core"),) * (n_params + n_outs),
                  out_specs=(PartitionSpec("core"),) * n_outs,
                  check_rep=False),
        donate_argnums=tuple(range(n_params, n_params + n_outs)),
        keep_unused=True)

    def run(in_maps):
        concat_in = [
            np.concatenate([in_maps[c][nm] for c in range(NCORES)], axis=0)
            for nm in in_names]
        concat_zeros = [np.zeros((NCORES * z.shape[0], *z.shape[1:]), z.dtype)
                        for z in zero_outs]
        outs = sharded(*concat_in, *concat_zeros)
        return [
            {nm: np.asarray(outs[i]).reshape(NCORES, *out_avals[i].shape)[c]
             for i, nm in enumerate(out_names)}
            for c in range(NCORES)]

    run.sharded = sharded
    run.in_names = in_names
    run.zero_outs = zero_outs
    _cache[key] = run
    return run


def kernel(q, k, scale, mask, attention):
    results = _get_runner()(_prep_core_inputs(q, k, mask, attention,
                                              prec=PREC))
    attn = np.empty((B, H, L, L), np.float32)
    for core in range(NCORES):
        o = results[core]["out"]                # (BH, 2, 128, L)
        for l in range(BH):
            f = 4 * core + l
            b, h = f // H, f % H
            attn[b, h, :128] = o[l, 0]
            attn[b, h, 128:] = o[l, 1]
    return attn


# revision 23
# speedup vs baseline: 1.0344x; 1.0344x over previous
"""GATv2 attention-score kernel for 8 Trainium2 NeuronCores.

Reference computation (per b, h):
    scores[i, j] = sum_d silu(q[i, d] + k[j, d]) * a[h, d]
    attn = softmax(where(mask, -FMAX, scores), axis=-1), zeroed at mask.

Sharding: the 32 (b, h) pairs are split 4-per-core (all four share one b,
so the mask is per-core constant).

Per-core dataflow (all shapes hardcoded: B=4, H=8, LQ=LK=256, D=64):
  - SBUF "summed" tile T (128, 8192): partitions = [d; d] (two stacked
    64-deep d blocks, one per query of a pair), free = 32 query-pairs x
    256 j.  Built with one DVE tensor_scalar_add per pair: in0 = kk
    (128, 256) = [k^T; k^T], scalar = per-partition column of qq
    (q values for the pair), which runs in the DVE 2x_2P mode.
  - ScalarE Silu over the whole tile (the compute floor of the problem:
    4 * 256 * 256 * 64 = 16.8M activations per core).
  - TensorE reduction over d: per pair one matmul with a sparse
    (128, 32) weight holding a_h in rows 0:64 of column 2m and rows
    64:128 of column 2m+1; 16 pair-matmuls accumulate into one
    32-partition PSUM strip (tile_position=(0, 32g)), four strips fill
    a (128, 256) PSUM tile = scores for one i-tile.
  - DVE evacuates PSUM + maskMIN (premasked float32.min addend) into an
    SBUF scores strip; after all 4 (b,h): one ScalarE Exp (masked
    entries underflow to exactly 0), DVE segment reduce_sum +
    reciprocal + per-segment scale, DMA out.

Softmax skips the max-subtraction: scores are bounded (|s| < ~60) so
exp cannot overflow, and masked entries are exactly 0.  Fully-masked
rows would yield NaN but do not occur (P ~ 2^-256 per row).

Precision (PREC below): the summed/silu stage runs in fp16 by default
(k, summed tile, reduction weights fp16; q enters as an exact fp32
per-partition scalar; PSUM/scores/softmax all fp32).  Measured on HW:
  PREC="fp16":  ~110-116 us/core/iter, max rel err 9.0e-4 (abs 1.5e-5)
  PREC="fp32":  ~199 us,               max rel err 1.3e-5 (abs 2.8e-7)
The kernel is ScalarE-bound; the theoretical silu floor is 109.2 us, so
fp16 is at ~101% of roofline (fp32 pays a DVE build-op-overhead wall).
Set PREC = "fp32" for a bit-conservative run if tighter accuracy is
ever required.
"""

import numpy as np

B, H, L, D = 4, 8, 256, 64
NCORES = 8
BH = 4          # (b, h) pairs per core
NPAIR = 128     # query pairs per (b, h)
TBLK = 64       # query pairs per summed tile (fp16; fp32 uses 32)
FT = TBLK * L   # summed tile free size (16384)
FMIN = np.float32(np.finfo(np.float32).min)

_cache = {}
PREC = "fp16"      # "fp32" | "fp16" | "mixed" — summed/silu stage dtype


def _build_program(reps=1, stages="full", prec="fp32", t_bufs=4, ps_bufs=4, tblk_ov=None):
    import concourse.mybir as mybir
    from concourse import bacc
    from concourse.tile import TileContext

    DT = mybir.dt.float32
    HT = mybir.dt.float32 if prec == "fp32" else mybir.dt.float16
    WT = DT if prec in ("fp32", "mixed") else HT    # matmul operand dtype
    nc = bacc.Bacc("TRN2", target_bir_lowering=False, debug=False,
                   num_devices=NCORES)

    kk_d = nc.dram_tensor("kk", [BH, 128, L], HT, kind="ExternalInput")
    qq_d = nc.dram_tensor("qq", [BH, 128, NPAIR], DT, kind="ExternalInput")
    wz_d = nc.dram_tensor("wz", [128, BH * 16 * 32], WT, kind="ExternalInput")
    mm_d = nc.dram_tensor("mm", [128, 2 * L], DT, kind="ExternalInput")
    out_d = nc.dram_tensor("out", [BH, 2, 128, L], DT, kind="ExternalOutput")

    with TileContext(nc) as tc:
        with (
            tc.tile_pool(name="io", bufs=2) as io_pool,
            tc.tile_pool(name="const", bufs=1) as c_pool,
            tc.tile_pool(name="summed", bufs=3) as t_pool,
            tc.tile_pool(name="psum", bufs=ps_bufs, space="PSUM") as ps_pool,
        ):
            wz_t = c_pool.tile([128, BH * 16 * 32], WT, tag="wz")
            nc.sync.dma_start(wz_t[:], wz_d[:])
            mm_t = c_pool.tile([128, 2 * L], DT, tag="mm")
            nc.sync.dma_start(mm_t[:], mm_d[:])
            scores = c_pool.tile([128, BH * 2 * L], DT, tag="scores")
            sums = c_pool.tile([128, BH * 2], DT, tag="sums")
            recip = c_pool.tile([128, BH * 2], DT, tag="recip")

            for _rep in range(reps):
                for l in range(BH):
                    kk_t = io_pool.tile([128, L], HT, tag="kk")
                    nc.sync.dma_start(kk_t[:], kk_d[l])
                    qq_t = io_pool.tile([128, NPAIR], DT, tag="qq")
                    nc.sync.dma_start(qq_t[:], qq_d[l])

                    tblk = tblk_ov or (32 if prec == "fp32" else TBLK)
                    ft = tblk * L
                    for it in range(2):      # i-tile = 128 queries
                        ps = ps_pool.tile([128, L], DT, tag="ps")
                        for tt in range(64 // tblk):
                            T = t_pool.tile([128, ft], HT, tag="T",
                                            bufs=t_bufs)
                            if prec == "mixed":
                                S = t_pool.tile([128, ft], WT, tag="S",
                                                name="S", bufs=1)
                            else:
                                S = T
                            base = it * 64 + tt * tblk  # first pair in tile
                            for blk in range(tblk):
                                c = base + blk
                                nc.vector.tensor_scalar_add(
                                    T[:, blk * L:(blk + 1) * L], kk_t[:],
                                    qq_t[:, c:c + 1])
                            if stages == "build":
                                continue
                            nc.scalar.activation(
                                S[:], T[:],
                                mybir.ActivationFunctionType.Silu)
                            if stages == "silu":
                                continue
                            for blk in range(tblk):
                                lc = tt * tblk + blk
                                g, m = lc // 16, lc % 16
                                nc.tensor.matmul(
                                    ps[32 * g:32 * g + 32, :],
                                    lhsT=wz_t[:, (l * 16 + m) * 32:
                                              (l * 16 + m + 1) * 32],
                                    rhs=S[:, blk * L:(blk + 1) * L],
                                    start=(m == 0), stop=(m == 15),
                                    tile_position=(0, 32 * g))
                        seg = l * 2 + it
                        if stages in ("build", "silu"):
                            continue
                        nc.vector.tensor_tensor(
                            scores[:, seg * L:(seg + 1) * L], ps[:],
                            mm_t[:, it * L:(it + 1) * L], mybir.AluOpType.add)

                if stages in ("build", "silu"):
                    # keep per-rep work observable: flush last T to out
                    if prec == "fp32":
                        nc.sync.dma_start(out_d[0, 0], T[:, :L])
                    else:
                        nc.sync.dma_start(out_d[0, 0, :, :L // 2],
                                          T[:, :L].bitcast(mybir.dt.float32))
                    continue
                if stages == "mm":
                    nc.sync.dma_start(out_d[0, 0], scores[:, :L])
                    continue
                nc.scalar.activation(scores[:], scores[:],
                                     mybir.ActivationFunctionType.Exp)
                nc.vector.reduce_sum(
                    sums[:], scores[:].rearrange("p (s j) -> p s j", j=L),
                    axis=mybir.AxisListType.X)
                nc.vector.reciprocal(recip[:], sums[:])
                for seg in range(BH * 2):
                    nc.vector.tensor_scalar_mul(
                        scores[:, seg * L:(seg + 1) * L],
                        scores[:, seg * L:(seg + 1) * L],
                        recip[:, seg:seg + 1])
                for l in range(BH):
                    for it in range(2):
                        seg = l * 2 + it
                        nc.sync.dma_start(out_d[l, it],
                                          scores[:, seg * L:(seg + 1) * L])

    nc.compile()
    return nc


def _prep_core_inputs(q, k, mask, attention, prec="fp32"):
    """Host-side layout prep: per-core input dicts."""
    ht = np.float32 if prec == "fp32" else np.float16
    wt = np.float32 if prec in ("fp32", "mixed") else np.float16
    q = np.asarray(q, np.float32)
    k = np.asarray(k, np.float32)
    a = np.asarray(attention, np.float32).reshape(H, D)
    mask = np.asarray(mask).reshape(B, L, L)

    in_maps = []
    for core in range(NCORES):
        kk = np.empty((BH, 128, L), ht)
        qq = np.empty((BH, 128, NPAIR), np.float32)
        wz4 = np.zeros((BH, 16, 128, 32), wt)
        for l in range(BH):
            f = 4 * core + l
            b, h = f // H, f % H
            kT = k[b, h].T                      # (D, L)
            kk[l, :64] = kT
            kk[l, 64:] = kT
            qq[l, :64] = q[b, h, 0::2].T        # even queries
            qq[l, 64:] = q[b, h, 1::2].T        # odd queries
            for m in range(16):
                wz4[l, m, :64, 2 * m] = a[h]
                wz4[l, m, 64:, 2 * m + 1] = a[h]
        wz = np.ascontiguousarray(
            wz4.transpose(2, 0, 1, 3).reshape(128, BH * 16 * 32))
        mb = np.where(mask[4 * core // H], FMIN, np.float32(0))
        mm = np.ascontiguousarray(
            np.concatenate([mb[:128], mb[128:]], axis=1).astype(np.float32))
        in_maps.append({"kk": kk, "qq": qq, "wz": wz, "mm": mm})
    return in_maps


def _get_runner(prec=None):
    """Persistent jitted shard_map runner over 8 cores.

    Mirrors concourse.bass2jax.run_bass_via_pjrt but caches the jitted
    callable so repeat kernel() calls skip retracing/recompiling.
    """
    if prec is None:
        prec = PREC
    key = ("runner", prec)
    if key in _cache:
        return _cache[key]

    import jax
    import concourse.mybir as mybir
    from jax.sharding import Mesh, PartitionSpec
    from jax.experimental.shard_map import shard_map
    from concourse import bass2jax

    bass2jax.install_neuronx_cc_hook()
    nc = _build_program(prec=prec)

    part_name = (nc.partition_id_tensor.name
                 if nc.partition_id_tensor else None)
    in_names, out_names, out_avals, zero_outs = [], [], [], []
    for alloc in nc.m.functions[0].allocations:
        if not isinstance(alloc, mybir.MemoryLocationSet):
            continue
        name = alloc.memorylocations[0].name
        if alloc.kind == "ExternalInput":
            if name != part_name:
                in_names.append(name)
        elif alloc.kind == "ExternalOutput":
            shape = tuple(alloc.tensor_shape)
            dtype = mybir.dt.np(alloc.dtype)
            out_names.append(name)
            out_avals.append(jax.core.ShapedArray(shape, dtype))
            zero_outs.append(np.zeros(shape, dtype))
    n_params = len(in_names)
    all_names = in_names + out_names
    if part_name is not None:
        all_names = all_names + [part_name]

    def _body(*args):
        operands = list(args)
        if part_name is not None:
            operands.append(bass2jax.partition_id_tensor())
        return tuple(bass2jax._bass_exec_p.bind(
            *operands,
            out_avals=tuple(out_avals),
            in_names=tuple(all_names),
            out_names=tuple(out_names),
            lowering_input_output_aliases=(),
            sim_require_finite=True,
            sim_require_nnan=True,
            nc=nc,
        ))

    devices = jax.devices()[:NCORES]
    mesh = Mesh(np.asarray(devices), ("core",))
    n_outs = len(out_names)
    sharded = jax.jit(
        shard_map(_body, mesh=mesh,
                  in_specs=(PartitionSpec("core"),) * (n_params + n_outs),
                  out_specs=(PartitionSpec("core"),) * n_outs,
                  check_rep=False),
        donate_argnums=tuple(range(n_params, n_params + n_outs)),
        keep_unused=True)

    def run(in_maps):
        concat_in = [
            np.concatenate([in_maps[c][nm] for c in range(NCORES)], axis=0)
            for nm in in_names]
        concat_zeros = [np.zeros((NCORES * z.shape[0], *z.shape[1:]), z.dtype)
                        for z in zero_outs]
        outs = sharded(*concat_in, *concat_zeros)
        return [
            {nm: np.asarray(outs[i]).reshape(NCORES, *out_avals[i].shape)[c]
             for i, nm in enumerate(out_names)}
            for c in range(NCORES)]

    run.sharded = sharded
    run.in_names = in_names
    run.zero_outs = zero_outs
    _cache[key] = run
    return run


def kernel(q, k, scale, mask, attention):
    results = _get_runner()(_prep_core_inputs(q, k, mask, attention,
                                              prec=PREC))
    attn = np.empty((B, H, L, L), np.float32)
    for core in range(NCORES):
        o = results[core]["out"]                # (BH, 2, 128, L)
        for l in range(BH):
            f = 4 * core + l
            b, h = f // H, f % H
            attn[b, h, :128] = o[l, 0]
            attn[b, h, 128:] = o[l, 1]
    return attn
